# revision 1
# baseline (speedup 1.0000x reference)
"""Trainium2 Bass kernel for nn_ConvAttention_34600256537137.

Math notes (validated against the reference):
  qkv = 1x1conv(x, w1)+b1 -> Q,K,V;  score = conv5x5(Q_s)+conv5x5(K_t)+b2;
  attn = softmax_t(score);  out = einsum(attn, V).
  Softmax over t is shift-invariant, so the Q-half of the score (constant in
  t), b2, and the K-path bias all cancel.  The computation collapses to:
    weff[ci,dy,dx] = sum_c w1K[c,ci] * w2K[c,dy,dx]        (host, tiny)
    sK[b,t,h,w]    = conv5x5_reflect(x[b,:,:,:,t], weff)
    e = exp(sK);  den = sum_t e
    out[b,o,h,w,s] = (sum_{ci,t} w1V[o,ci] * e * x) / den + b1V[o]
  (s-independent; normalization folded to the end; bias + S-broadcast on host)

Sharding: 8 cores = (b in {0,1}) x (4 chunks of 8 rows of H).

Perf structure (v5; bf16 datapath, rel err ~4e-3 vs 2e-2 budget):
  - conv contraction folded over (ci, t%2): K=128, M=50 (tap x t-parity with
    zero padding), so 12 row-matmuls of 288 cols replace 24 and PSUM->SBUF
    copies run on 50 lanes instead of 25.
  - T lands in DRAM dy-pre-shifted (8-row windows) so (t2,h) merges into one
    stride-36 dim and each (dy, tp) gather is a legal 3-dim DMA: 5 writes +
    10 gathers instead of 25; DVE reduces the 25 taps on 128 lanes.
  - softmax denominator via indicator-matmul on PE; e replicated to the
    (ci8,t)-partition layout with two parallel 0-stride-AP DMAs.
  - xattn split across vector/gpsimd; V matmuls pipeline behind it.
  - device emits only the s-independent [C, HW] tile; the S broadcast and
    +b1V happen on host.
"""

import sys

if "/opt/trn_rl_repo" not in sys.path:
    sys.path.insert(0, "/opt/trn_rl_repo")

import numpy as np
import ml_dtypes

BF16 = ml_dtypes.bfloat16

B, C, H, W, S = 2, 64, 32, 32, 16
KS, PAD = 5, 2
NCORES = 8
ROWS = H // 4            # output rows per core
SLAB_R = ROWS + 2 * PAD  # 12
SLAB_W = W + 2 * PAD     # 36
NTAP = KS * KS           # 25
HW = ROWS * W            # 256
S2 = S // 2              # 8 frame-pairs
M2 = 2 * NTAP            # 50 stationary columns (tap, t-parity)
TAPW = S2 * ROWS * SLAB_W  # 2304: td elements per (tap, tp) plane

_MODULE = None


def _build_module():
    import concourse.bacc as bacc
    import concourse.bass as bass
    import concourse.tile as tile
    from concourse import mybir

    f32 = mybir.dt.float32
    bf16 = mybir.dt.bfloat16
    AF = mybir.ActivationFunctionType
    ALU = mybir.AluOpType
    nc = bacc.Bacc("TRN2", target_bir_lowering=False, debug=False, num_devices=NCORES)

    # slab partitions are (ci, t%2); innermost frame axis is t2 = t//2
    slab_d = nc.dram_tensor("slab", [128, SLAB_R, SLAB_W, S2], bf16, kind="ExternalInput")
    xt_d = nc.dram_tensor("xt", [128, 8, HW], bf16, kind="ExternalInput")
    weff_d = nc.dram_tensor("weff", [128, M2], bf16, kind="ExternalInput")
    w1vr_d = nc.dram_tensor("w1vr", [128, 8, C], bf16, kind="ExternalInput")
    hsel_d = nc.dram_tensor("hsel", [128, ROWS], bf16, kind="ExternalInput")
    o_d = nc.dram_tensor("o", [C, HW], f32, kind="ExternalOutput")
    dn_d = nc.dram_tensor("dn", [ROWS * W], f32, kind="ExternalOutput")

    # scratch DRAM for partition-crossing rearrangements.
    # td holds T with rows pre-shifted by each tap's dy (8-row windows), so
    # (t2,h) merges into a single stride-36 dim and each (dy,tp) gather is a
    # legal 3-dim DMA pattern.
    td_d = nc.dram_tensor("td", [M2, S2, ROWS, SLAB_W], bf16)
    ed_d = nc.dram_tensor("ed", [S, ROWS, W], bf16)               # exp(sK), frame-major

    with tile.TileContext(nc) as tc:
        with tc.tile_pool(name="sb", bufs=1) as sb, tc.tile_pool(
            name="ps", bufs=6, space="PSUM"
        ) as ps, tc.tile_pool(name="pso", bufs=1, space="PSUM") as pso:
            # --- loads: slab split as single rows up front (fast first
            # matmul) then two-row chunks round-robin.  xt/w1vr are deferred
            # to gpsimd's idle mid-kernel window so the slab and T traffic
            # own clean queues. ---
            s_weff = sb.tile([128, M2], bf16)
            nc.gpsimd.dma_start(s_weff, weff_d.ap())
            s_hsel = sb.tile([128, ROWS], bf16)
            nc.gpsimd.dma_start(s_hsel, hsel_d.ap())
            s_slab = sb.tile([128, SLAB_R, SLAB_W, S2], bf16)
            chunks = (
                (0, 1, nc.sync), (1, 2, nc.scalar), (2, 4, nc.gpsimd),
                (4, 6, nc.sync), (6, 8, nc.scalar), (8, 10, nc.gpsimd),
                (10, 12, nc.sync),
            )
            for r0, r1, e in chunks:
                e.dma_start(
                    s_slab[:, r0:r1, :, :], slab_d.ap()[:, r0:r1, :, :]
                )
            # xt/w1vr tiles are loaded later, between the T writebacks, so
            # their bulk transfers don't steal DMA-pool bandwidth from the
            # slab chunks that feed the conv matmuls
            s_xt = sb.tile([128, 8, HW], bf16)
            s_w1vr = sb.tile([128, 8, C], bf16)

            # --- phase A: T[(tap,tp), (w,t2)] = weff^T @ slab, one matmul per
            # slab row; copies transpose to t2-major (tap', t2, row, w) ---
            s_T = sb.tile([M2, S2, SLAB_R, SLAB_W], bf16)
            for row in range(SLAB_R):
                p_t = ps.tile([M2, SLAB_W, S2], f32, tag="pt")
                nc.tensor.matmul(
                    p_t, s_weff, s_slab[:, row, :, :], start=True, stop=True
                )
                dst = s_T[:, :, row, :]
                if row % 2 == 0:
                    nc.scalar.copy(dst, p_t.transpose([0, 2, 1]))
                else:
                    nc.vector.tensor_copy(dst, p_t.transpose([0, 2, 1]))

            # --- T to DRAM: 5 dy-class writes of dy-shifted 8-row windows,
            # each followed by its two (tp) 3-dim batched gathers into
            # R[(tp,t2,h), (dy,dx), w].  Write dy depends on copies for rows
            # dy..dy+7 only, so early dy classes pipeline behind the conv. ---
            s_R = sb.tile([128, NTAP, W], bf16)
            g_engs = (nc.sync, nc.sync, nc.scalar, nc.sync, nc.scalar)
            for dy in range(KS):
                e = g_engs[dy]
                e.dma_start(
                    td_d.ap()[2 * KS * dy : 2 * KS * dy + 2 * KS],
                    s_T[2 * KS * dy : 2 * KS * dy + 2 * KS, :, dy : dy + ROWS, :],
                )
                for tp in range(2):
                    src = bass.AP(
                        tensor=td_d.ap().tensor,
                        offset=2 * KS * TAPW * dy + TAPW * tp,
                        ap=[[SLAB_W, S2 * ROWS], [2 * TAPW + 1, KS], [1, W]],
                    )
                    e.dma_start(
                        s_R[64 * tp : 64 * tp + 64, KS * dy : KS * dy + KS, :], src
                    )
                if dy == 1:
                    # gpsimd (slow software DGE) carries only this small load
                    nc.gpsimd.dma_start(s_w1vr, w1vr_d.ap())

            # xt halves on the fast HW-DGE queues right after their phase-B
            # work: transfers land well before xattn needs them
            nc.sync.dma_start(s_xt[:, 0:4, :], xt_d.ap()[:, 0:4, :])
            nc.scalar.dma_start(s_xt[:, 4:8, :], xt_d.ap()[:, 4:8, :])


            # --- tap reduce on 128 lanes (strided view puts tap innermost) ---
            s_sk = sb.tile([128, W], f32)  # [(tp,t2,h), w]
            nc.vector.tensor_reduce(
                s_sk, s_R.transpose([0, 2, 1]), axis=mybir.AxisListType.X, op=ALU.add
            )

            # --- e = exp(sK); den via indicator-matmul on PE ---
            s_e16 = sb.tile([128, W], bf16)
            nc.scalar.activation(s_e16, s_sk, AF.Exp)
            p_den = pso.tile([ROWS, W], f32, tag="den")
            nc.tensor.matmul(p_den, s_hsel, s_e16, start=True, stop=True)
            s_rcp = sb.tile([ROWS, W], f32)
            nc.vector.reciprocal(s_rcp, p_den)
            nc.gpsimd.dma_start(dn_d.ap(), s_rcp)

            # --- bounce e to frame-major [t, hw] (2 writes, one per parity);
            # replicate to [(ci8,t), hw] with two parallel 0-stride reads ---
            for tp, e in ((0, nc.scalar), (1, nc.sync)):
                e.dma_start(
                    bass.AP(
                        tensor=ed_d.ap().tensor,
                        offset=HW * tp,
                        ap=[[2 * HW, S2], [W, ROWS], [1, W]],
                    ),
                    s_e16[64 * tp : 64 * tp + 64, :],
                )
            s_eb = sb.tile([128, HW], bf16)
            for half, e in ((0, nc.scalar), (1, nc.sync)):
                e.dma_start(
                    s_eb[64 * half : 64 * half + 64, :],
                    bass.AP(
                        tensor=ed_d.ap().tensor,
                        offset=0,
                        ap=[[0, 4], [HW, S], [1, HW]],
                    ),
                )

            # --- V path: xattn = x_t * e in four chunks so the V matmuls
            # pipeline tightly behind the multiplies ---
            s_xa = sb.tile([128, 8, HW], bf16)
            ebb = s_eb.unsqueeze(1).broadcast_to((128, 2, HW))
            p_o = pso.tile([C, HW], f32, tag="out")
            for q in range(4):
                nc.vector.tensor_tensor(
                    s_xa[:, 2 * q : 2 * q + 2, :],
                    s_xt[:, 2 * q : 2 * q + 2, :],
                    ebb,
                    op=ALU.mult,
                )
            for g in range(8):
                nc.tensor.matmul(
                    p_o,
                    s_w1vr[:, g, :],
                    s_xa[:, g, :],
                    start=(g == 0),
                    stop=(g == 7),
                )
            # raw (unnormalized) output; the host divides by den via dn
            s_o = sb.tile([C, HW], f32)
            nc.vector.tensor_copy(s_o, p_o)
            nc.sync.dma_start(o_d.ap(), s_o)

    nc.compile()
    return nc


def _get_module():
    global _MODULE
    if _MODULE is None:
        _MODULE = _build_module()
    return _MODULE


def make_host_inputs(x, w1, b1, w2, b2):
    """Host-side precompute: folded weights + per-core reflect-padded slices."""
    x = np.ascontiguousarray(np.asarray(x, np.float32))
    w1 = np.asarray(w1, np.float32)
    w2 = np.asarray(w2, np.float32)

    w1K = w1[C : 2 * C, :, 0, 0]          # [c, ci]
    w2K = w2[0, C : 2 * C]                # [c, 5, 5]
    weff = np.einsum("ci,cyx->iyx", w1K, w2K).reshape(C, NTAP)
    # weff2[(ci,tp), (tap,tpo)] = weff[ci, tap] * [tpo == tp]
    weff2 = np.zeros((128, M2), np.float32)
    for tp in range(2):
        weff2[tp::2, tp::2] = weff
    weff2 = weff2.astype(BF16)
    w1V = w1[2 * C :, :, 0, 0]            # [co, ci]

    # w1vr[(ci8,t), g, co] = w1V[co, 8g+ci8]
    tmp = w1V.T.reshape(8, 8, C)                      # (g, ci8, co)
    w1vr = np.ascontiguousarray(
        np.broadcast_to(tmp[:, :, None, :], (8, 8, S, C))
        .transpose(1, 2, 0, 3)
        .reshape(128, 8, C)
    ).astype(BF16)

    # hsel[(tp,t2,h), m] = 1 if h == m
    hsel = np.zeros((128, ROWS), np.float32)
    for p in range(128):
        hsel[p, p % ROWS] = 1.0
    hsel = hsel.astype(BF16)

    in_maps = []
    for core in range(NCORES):
        b, hc = divmod(core, 4)
        h0 = ROWS * hc
        xp = np.pad(x[b], ((0, 0), (PAD, PAD), (PAD, PAD), (0, 0)), mode="reflect")
        sl = xp[:, h0 : h0 + SLAB_R, :, :]            # [ci, r, w36, t]
        slab = np.ascontiguousarray(
            sl.reshape(C, SLAB_R, SLAB_W, S2, 2)
            .transpose(0, 4, 1, 2, 3)
            .reshape(128, SLAB_R, SLAB_W, S2)
        ).astype(BF16)
        xs = x[b][:, h0 : h0 + ROWS, :, :]            # [ci, h, w, t]
        xt = np.ascontiguousarray(
            xs.reshape(8, 8, ROWS, W, S)
            .transpose(1, 4, 0, 2, 3)
            .reshape(128, 8, HW)
        ).astype(BF16)
        in_maps.append(
            {"slab": slab, "xt": xt, "weff": weff2, "w1vr": w1vr, "hsel": hsel}
        )
    return in_maps


def assemble_output(results, b1):
    b1V = np.asarray(b1, np.float32)[2 * C :]
    out = np.empty((B, C, H, W, S), np.float32)
    for core in range(NCORES):
        b, hc = divmod(core, 4)
        h0 = ROWS * hc
        r = results[core]
        o = r["o"].reshape(C, ROWS, W) * r["dn"].reshape(1, ROWS, W)
        out[b, :, h0 : h0 + ROWS, :, :] = o[:, :, :, None]
    out += b1V[None, :, None, None, None]
    return out


def kernel(x, w1, b1, w2, b2):
    from concourse.bass_utils import run_bass_kernel_spmd

    nc = _get_module()
    in_maps = make_host_inputs(x, w1, b1, w2, b2)
    res = run_bass_kernel_spmd(nc, in_maps, core_ids=list(range(NCORES)))
    return assemble_output(res.results, b1)



# revision 4
# speedup vs baseline: 1.0802x; 1.0802x over previous
"""Trainium2 Bass kernel for nn_ConvAttention_34600256537137.

Math notes (validated against the reference):
  qkv = 1x1conv(x, w1)+b1 -> Q,K,V;  score = conv5x5(Q_s)+conv5x5(K_t)+b2;
  attn = softmax_t(score);  out = einsum(attn, V).
  Softmax over t is shift-invariant, so the Q-half of the score (constant in
  t), b2, and the K-path bias all cancel.  The computation collapses to:
    weff[ci,dy,dx] = sum_c w1K[c,ci] * w2K[c,dy,dx]        (host, tiny)
    sK[b,t,h,w]    = conv5x5_reflect(x[b,:,:,:,t], weff)
    e = exp(sK);  den = sum_t e
    out[b,o,h,w,s] = (sum_{ci,t} w1V[o,ci] * e * x) / den + b1V[o]
  (s-independent; normalization folded to the end; bias + S-broadcast on host)

Sharding: 8 cores = (b in {0,1}) x (4 chunks of 8 rows of H).

Perf structure (v6; bf16 datapath):
  - conv contraction folded over (ci, t%2): K=128, M=50 (tap x t-parity with
    zero padding), 12 row-matmuls of 288 cols; PSUM->SBUF copies on 50 lanes.
  - slab rides ONLY the two HWDGE rings (sync/scalar) in 2 chunks each; the
    software (gpsimd) queue carries nothing hot.
  - T lands in DRAM dy-pre-shifted (8-row windows) so (t2,h) merges into one
    stride-36 dim and each (dy, tp) gather is a legal 3-dim DMA: 5 writes +
    10 gathers; per-dy partial tap reduces hide behind the gather DMAs.
  - softmax denominator via indicator-matmul on PE.
  - e replicated to the (ci8,t)-partition layout ON-CHIP: mask-multiply
    expands e16 to a (partition-diagonal) [128, 8x32] tile, one indicator
    matmul (esel) emits eb[(rep,t),(h,w)] in PSUM; no DRAM round trip.
  - xt/w1vr are queued on the HWDGE rings AFTER the gathers so the gather
    packets own the rings mid-kernel; they land during the softmax phase.
  - xattn split across vector ops; the V matmuls pipeline behind it; output
    is emitted bf16 (error budget 2e-2 >> bf16 rounding).
"""

import sys

if "/opt/trn_rl_repo" not in sys.path:
    sys.path.insert(0, "/opt/trn_rl_repo")

import numpy as np
import ml_dtypes

BF16 = ml_dtypes.bfloat16

B, C, H, W, S = 2, 64, 32, 32, 16
KS, PAD = 5, 2
NCORES = 8
ROWS = H // 4            # output rows per core
SLAB_R = ROWS + 2 * PAD  # 12
SLAB_W = W + 2 * PAD     # 36
NTAP = KS * KS           # 25
HW = ROWS * W            # 256
S2 = S // 2              # 8 frame-pairs
M2 = 2 * NTAP            # 50 stationary columns (tap, t-parity)
TAPW = S2 * ROWS * SLAB_W  # 2304: td elements per (tap, tp) plane

_MODULE = None


def _build_module():
    import concourse.bacc as bacc
    import concourse.bass as bass
    import concourse.tile as tile
    from concourse import mybir

    f32 = mybir.dt.float32
    bf16 = mybir.dt.bfloat16
    AF = mybir.ActivationFunctionType
    ALU = mybir.AluOpType
    nc = bacc.Bacc("TRN2", target_bir_lowering=False, debug=False, num_devices=NCORES)

    # slab partitions are (ci, t%2); innermost frame axis is t2 = t//2
    slab_d = nc.dram_tensor("slab", [128, SLAB_R, SLAB_W, S2], bf16, kind="ExternalInput")
    xt_d = nc.dram_tensor("xt", [128, 8, HW], bf16, kind="ExternalInput")
    weff_d = nc.dram_tensor("weff", [128, M2], bf16, kind="ExternalInput")
    w1vr_d = nc.dram_tensor("w1vr", [128, 8, C], bf16, kind="ExternalInput")
    hsel_d = nc.dram_tensor("hsel", [128, ROWS], bf16, kind="ExternalInput")
    esel_d = nc.dram_tensor("esel", [128, 128], bf16, kind="ExternalInput")
    hmask_d = nc.dram_tensor("hmask", [128, ROWS], bf16, kind="ExternalInput")
    o_d = nc.dram_tensor("o", [C, HW], bf16, kind="ExternalOutput")
    dn_d = nc.dram_tensor("dn", [ROWS * W], f32, kind="ExternalOutput")

    # scratch DRAM for the partition-crossing tap gather.
    # td holds T with rows pre-shifted by each tap's dy (8-row windows), so
    # (t2,h) merges into a single stride-36 dim and each (dy,tp) gather is a
    # legal 3-dim DMA pattern.
    td_d = nc.dram_tensor("td", [M2, S2, ROWS, SLAB_W], bf16)

    with tile.TileContext(nc) as tc:
        with tc.tile_pool(name="sb", bufs=1) as sb, tc.tile_pool(
            name="ps", bufs=4, space="PSUM"
        ) as ps, tc.tile_pool(name="pso", bufs=1, space="PSUM") as pso:
            # --- small stationary tensors first (needed by the first matmul
            # and by the softmax phase); they are tiny so they clear the sync
            # ring almost immediately. ---
            s_weff = sb.tile([128, M2], bf16)
            nc.sync.dma_start(s_weff, weff_d.ap())
            s_hsel = sb.tile([128, ROWS], bf16)
            s_esel = sb.tile([128, 128], bf16)
            s_hmask = sb.tile([128, ROWS], bf16)
            nc.scalar.dma_start(s_hsel, hsel_d.ap())
            nc.scalar.dma_start(s_esel, esel_d.ap())
            nc.scalar.dma_start(s_hmask, hmask_d.ap())

            # --- slab: 2 chunks per HWDGE ring; single leading rows so the
            # first matmuls start early, bulk rows behind them. ---
            s_slab = sb.tile([128, SLAB_R, SLAB_W, S2], bf16)
            chunks = (
                (0, 1, nc.sync), (1, 2, nc.scalar),
                (2, 7, nc.sync), (7, 12, nc.scalar),
            )
            for r0, r1, e in chunks:
                e.dma_start(
                    s_slab[:, r0:r1, :, :], slab_d.ap()[:, r0:r1, :, :]
                )
            s_xt = sb.tile([128, 8, HW], bf16)
            s_w1vr = sb.tile([128, 8, C], bf16)

            # --- phase A: T[(tap,tp), (w,t2)] = weff^T @ slab, one matmul per
            # slab row; copies transpose to t2-major (tap', t2, row, w) ---
            s_T = sb.tile([M2, S2, SLAB_R, SLAB_W], bf16)
            for row in range(SLAB_R):
                p_t = ps.tile([M2, SLAB_W, S2], f32, tag="pt")
                nc.tensor.matmul(
                    p_t, s_weff, s_slab[:, row, :, :], start=True, stop=True
                )
                dst = s_T[:, :, row, :]
                if row % 2 == 0:
                    nc.scalar.copy(dst, p_t.transpose([0, 2, 1]))
                else:
                    nc.vector.tensor_copy(dst, p_t.transpose([0, 2, 1]))

            # --- T to DRAM: 5 dy-class writes of dy-shifted 8-row windows,
            # each followed by its two (tp) 3-dim batched gathers into
            # R[(tp,t2,h), (dy,dx), w].  Write dy depends on copies for rows
            # dy..dy+7 only, so early dy classes pipeline behind the conv.
            # A partial tap-reduce runs on vector as each dy class lands, so
            # only the last dy's 5-tap reduce sits on the critical path. ---
            s_R = sb.tile([128, NTAP, W], bf16)
            s_par = sb.tile([128, KS, W], f32)  # per-dy partial sums
            g_engs = (nc.sync, nc.scalar, nc.sync, nc.scalar, nc.sync)
            for dy in range(KS):
                e = g_engs[dy]
                e.dma_start(
                    td_d.ap()[2 * KS * dy : 2 * KS * dy + 2 * KS],
                    s_T[2 * KS * dy : 2 * KS * dy + 2 * KS, :, dy : dy + ROWS, :],
                )
                for tp in range(2):
                    src = bass.AP(
                        tensor=td_d.ap().tensor,
                        offset=2 * KS * TAPW * dy + TAPW * tp,
                        ap=[[SLAB_W, S2 * ROWS], [2 * TAPW + 1, KS], [1, W]],
                    )
                    e.dma_start(
                        s_R[64 * tp : 64 * tp + 64, KS * dy : KS * dy + KS, :], src
                    )
                # partial reduce of this dy's 5 taps (vector, hides behind
                # the next dy's gather DMAs)
                nc.vector.tensor_reduce(
                    s_par[:, dy, :],
                    s_R[:, KS * dy : KS * dy + KS, :].transpose([0, 2, 1]),
                    axis=mybir.AxisListType.X,
                    op=ALU.add,
                )

            # xt/w1vr on the HWDGE rings behind the gathers: they land during
            # the softmax phase, well before xattn/V need them
            nc.sync.dma_start(s_xt[:, 0:4, :], xt_d.ap()[:, 0:4, :])
            nc.scalar.dma_start(s_xt[:, 4:8, :], xt_d.ap()[:, 4:8, :])
            nc.scalar.dma_start(s_w1vr, w1vr_d.ap())

            # --- final 5-way add of the partials -> sK ---
            s_sk = sb.tile([128, W], f32)  # [(tp,t2,h), w]
            nc.vector.tensor_reduce(
                s_sk, s_par.transpose([0, 2, 1]), axis=mybir.AxisListType.X, op=ALU.add
            )

            # --- e = exp(sK); den via indicator-matmul on PE ---
            s_e16 = sb.tile([128, W], bf16)
            nc.scalar.activation(s_e16, s_sk, AF.Exp)
            p_den = pso.tile([ROWS, W], f32, tag="den")
            nc.tensor.matmul(p_den, s_hsel, s_e16, start=True, stop=True)
            s_rcp = sb.tile([ROWS, W], f32)
            nc.vector.reciprocal(s_rcp, p_den)
            nc.gpsimd.dma_start(dn_d.ap(), s_rcp)

            # --- replicate e to the (ci8,t)-partition layout on-chip:
            # mask-multiply expands e16 to the partition-diagonal
            # [128, (h', w)] tile, then one indicator matmul emits
            # eb[(rep,t), (h,w)] in PSUM (no DRAM bounce). ---
            s_ed = sb.tile([128, ROWS, W], bf16)
            nc.vector.tensor_tensor(
                s_ed,
                s_e16.unsqueeze(1).broadcast_to((128, ROWS, W)),
                s_hmask.unsqueeze(2).broadcast_to((128, ROWS, W)),
                op=ALU.mult,
            )
            p_eb = pso.tile([128, HW], f32, tag="eb")
            nc.tensor.matmul(
                p_eb, s_esel, s_ed.rearrange("p a b -> p (a b)"), start=True, stop=True
            )
            s_eb = sb.tile([128, HW], bf16)
            nc.scalar.copy(s_eb, p_eb)

            # --- V path: xattn = x_t * e in four chunks so the V matmuls
            # pipeline tightly behind the multiplies ---
            s_xa = sb.tile([128, 8, HW], bf16)
            ebb = s_eb.unsqueeze(1).broadcast_to((128, 2, HW))
            p_o = pso.tile([C, HW], f32, tag="out")
            for q in range(4):
                nc.vector.tensor_tensor(
                    s_xa[:, 2 * q : 2 * q + 2, :],
                    s_xt[:, 2 * q : 2 * q + 2, :],
                    ebb,
                    op=ALU.mult,
                )
            for g in range(8):
                nc.tensor.matmul(
                    p_o,
                    s_w1vr[:, g, :],
                    s_xa[:, g, :],
                    start=(g == 0),
                    stop=(g == 7),
                )
            # raw (unnormalized) output; the host divides by den via dn
            s_o = sb.tile([C, HW], bf16)
            nc.vector.tensor_copy(s_o, p_o)
            nc.sync.dma_start(o_d.ap(), s_o)

    nc.compile()
    return nc


def _get_module():
    global _MODULE
    if _MODULE is None:
        _MODULE = _build_module()
    return _MODULE


def make_host_inputs(x, w1, b1, w2, b2):
    """Host-side precompute: folded weights + per-core reflect-padded slices."""
    x = np.ascontiguousarray(np.asarray(x, np.float32))
    w1 = np.asarray(w1, np.float32)
    w2 = np.asarray(w2, np.float32)

    w1K = w1[C : 2 * C, :, 0, 0]          # [c, ci]
    w2K = w2[0, C : 2 * C]                # [c, 5, 5]
    weff = np.einsum("ci,cyx->iyx", w1K, w2K).reshape(C, NTAP)
    # weff2[(ci,tp), (tap,tpo)] = weff[ci, tap] * [tpo == tp]
    weff2 = np.zeros((128, M2), np.float32)
    for tp in range(2):
        weff2[tp::2, tp::2] = weff
    weff2 = weff2.astype(BF16)
    w1V = w1[2 * C :, :, 0, 0]            # [co, ci]

    # w1vr[(ci8,t), g, co] = w1V[co, 8g+ci8]
    tmp = w1V.T.reshape(8, 8, C)                      # (g, ci8, co)
    w1vr = np.ascontiguousarray(
        np.broadcast_to(tmp[:, :, None, :], (8, 8, S, C))
        .transpose(1, 2, 0, 3)
        .reshape(128, 8, C)
    ).astype(BF16)

    # hsel[(tp,t2,h), m] = 1 if h == m   (den indicator matmul)
    hsel = np.zeros((128, ROWS), np.float32)
    for p in range(128):
        hsel[p, p % ROWS] = 1.0
    hsel = hsel.astype(BF16)
    # hmask == hsel (diagonal h-expansion mask for the eb build)
    hmask = hsel

    # esel[(tp,t2,h), (rep,t')] = 1 if t' == 2*t2+tp   (eb indicator matmul)
    esel = np.zeros((128, 128), np.float32)
    for p in range(128):
        tp, t2 = p // 64, (p % 64) // ROWS
        t = 2 * t2 + tp
        for rep in range(8):
            esel[p, rep * S + t] = 1.0
    esel = esel.astype(BF16)

    in_maps = []
    for core in range(NCORES):
        b, hc = divmod(core, 4)
        h0 = ROWS * hc
        xp = np.pad(x[b], ((0, 0), (PAD, PAD), (PAD, PAD), (0, 0)), mode="reflect")
        sl = xp[:, h0 : h0 + SLAB_R, :, :]            # [ci, r, w36, t]
        slab = np.ascontiguousarray(
            sl.reshape(C, SLAB_R, SLAB_W, S2, 2)
            .transpose(0, 4, 1, 2, 3)
            .reshape(128, SLAB_R, SLAB_W, S2)
        ).astype(BF16)
        xs = x[b][:, h0 : h0 + ROWS, :, :]            # [ci, h, w, t]
        xt = np.ascontiguousarray(
            xs.reshape(8, 8, ROWS, W, S)
            .transpose(1, 4, 0, 2, 3)
            .reshape(128, 8, HW)
        ).astype(BF16)
        in_maps.append(
            {
                "slab": slab,
                "xt": xt,
                "weff": weff2,
                "w1vr": w1vr,
                "hsel": hsel,
                "esel": esel,
                "hmask": hmask,
            }
        )
    return in_maps


def assemble_output(results, b1):
    b1V = np.asarray(b1, np.float32)[2 * C :]
    out = np.empty((B, C, H, W, S), np.float32)
    for core in range(NCORES):
        b, hc = divmod(core, 4)
        h0 = ROWS * hc
        r = results[core]
        o = r["o"].astype(np.float32).reshape(C, ROWS, W) * r["dn"].reshape(
            1, ROWS, W
        )
        out[b, :, h0 : h0 + ROWS, :, :] = o[:, :, :, None]
    out += b1V[None, :, None, None, None]
    return out


def kernel(x, w1, b1, w2, b2):
    from concourse.bass_utils import run_bass_kernel_spmd

    nc = _get_module()
    in_maps = make_host_inputs(x, w1, b1, w2, b2)
    res = run_bass_kernel_spmd(nc, in_maps, core_ids=list(range(NCORES)))
    return assemble_output(res.results, b1)


# revision 6
# speedup vs baseline: 1.2260x; 1.1350x over previous
"""Trainium2 Bass kernel for nn_ConvAttention_34600256537137.

Math notes (validated against the reference):
  qkv = 1x1conv(x, w1)+b1 -> Q,K,V;  score = conv5x5(Q_s)+conv5x5(K_t)+b2;
  attn = softmax_t(score);  out = einsum(attn, V).
  Softmax over t is shift-invariant, so the Q-half of the score (constant in
  t), b2, and the K-path bias all cancel.  The computation collapses to:
    weff[ci,dy,dx] = sum_c w1K[c,ci] * w2K[c,dy,dx]        (host, tiny)
    sK[b,t,h,w]    = conv5x5_reflect(x[b,:,:,:,t], weff)
    e = exp(sK);  den = sum_t e
    out[b,o,h,w,s] = (sum_{ci,t} w1V[o,ci] * e * x) / den + b1V[o]
  (s-independent; normalization folded to the end; bias + S-broadcast on host)

Sharding: 8 cores = (b in {0,1}) x (4 chunks of 8 rows of H).

Perf structure (v7; bf16 datapath, row-PAIR conv folding):
  - conv contraction folds (ci, row-parity rp): K=128, stationary columns
    (po, par, dx) with weff3[(ci,rp), (po,par,dx)] = weff[ci, 2po+rp-par, dx]
    (out-of-range dy -> 0).  12 matmuls (6 slab row-pairs x 2 t-halves) of
    288 cols -- same PE cycles as the dy scheme but only THREE tap classes
    (po) survive to the bounce: 3 td writes + 6 gathers (vs 5+10), 1920
    gather descriptors (vs 3200), 15-tap reduce (vs 25).
  - slab is loaded in 3 big contiguous chunks (128 descriptors each) on the
    two HWDGE rings; the 4 constant tensors ride ONE consolidated DMA.
  - per-po partial tap reduces hide behind the gather DMAs.
  - softmax denominator via indicator-matmul on PE; e replicated to the
    (ci8,t)-partition layout ON-CHIP (mask-multiply diagonal expansion +
    esel indicator matmul) -- no DRAM round trip.
  - xt/w1vr queue on the HWDGE rings behind the gathers; xattn splits in 4
    chunks; V matmuls pipeline behind it; output is emitted bf16.
  sK partition layout: p = 64*par + 4*t + q0, output row h = 2*q0 + par.
"""

import sys

if "/opt/trn_rl_repo" not in sys.path:
    sys.path.insert(0, "/opt/trn_rl_repo")

import numpy as np
import ml_dtypes

BF16 = ml_dtypes.bfloat16

B, C, H, W, S = 2, 64, 32, 32, 16
KS, PAD = 5, 2
NCORES = 8
ROWS = H // 4            # output rows per core
SLAB_R = ROWS + 2 * PAD  # 12
NPAIR = SLAB_R // 2      # 6 slab row-pairs
SLAB_W = W + 2 * PAD     # 36
HW = ROWS * W            # 256
NPO = 3                  # pair-offset classes
M3 = NPO * 2 * KS        # 30 stationary columns (po, par, dx)
NTAP3 = NPO * KS         # 15 surviving tap classes
PLANE = S * 4 * SLAB_W   # 2304: td elements per (po,par,dx) plane
CONSTW = M3 + ROWS + ROWS + 128  # consolidated consts columns

_MODULE = None


def _build_module():
    import concourse.bacc as bacc
    import concourse.bass as bass
    import concourse.tile as tile
    from concourse import mybir

    f32 = mybir.dt.float32
    bf16 = mybir.dt.bfloat16
    AF = mybir.ActivationFunctionType
    ALU = mybir.AluOpType
    nc = bacc.Bacc("TRN2", target_bir_lowering=False, debug=False, num_devices=NCORES)

    # slab partitions are (ci, row-parity rp); free = (pair q, w', t)
    slab_d = nc.dram_tensor("slab", [128, NPAIR, SLAB_W, S], bf16, kind="ExternalInput")
    xt_d = nc.dram_tensor("xt", [128, 8, HW], bf16, kind="ExternalInput")
    cst_d = nc.dram_tensor("cst", [128, CONSTW], bf16, kind="ExternalInput")
    w1vr_d = nc.dram_tensor("w1vr", [128, 8, C], bf16, kind="ExternalInput")
    o_d = nc.dram_tensor("o", [C, HW], bf16, kind="ExternalOutput")
    dn_d = nc.dram_tensor("dn", [ROWS * W], f32, kind="ExternalOutput")

    # scratch DRAM for the partition-crossing tap gather.
    # td holds T3 with pair-rows pre-shifted by po (4-pair windows), so
    # (t, q0) merges into a single stride-36 dim and each (po, par) gather
    # is a legal 3-dim DMA pattern.
    td_d = nc.dram_tensor("td", [M3, S, 4, SLAB_W], bf16)

    with tile.TileContext(nc) as tc:
        with tc.tile_pool(name="sb", bufs=1) as sb, tc.tile_pool(
            name="ps", bufs=4, space="PSUM"
        ) as ps, tc.tile_pool(name="pso", bufs=1, space="PSUM") as pso:
            # --- one consolidated constants load (weff3 | hsel | hmask | esel)
            s_cst = sb.tile([128, CONSTW], bf16)
            nc.sync.dma_start(s_cst, cst_d.ap())
            s_weff = s_cst[:, 0:M3]
            s_hsel = s_cst[:, M3 : M3 + ROWS]
            s_hmask = s_cst[:, M3 + ROWS : M3 + 2 * ROWS]
            s_esel = s_cst[:, M3 + 2 * ROWS : M3 + 2 * ROWS + 128]

            # --- slab: 3 big contiguous chunks (pair 0 first so the matmuls
            # start early), split across the two HWDGE rings ---
            s_slab = sb.tile([128, NPAIR, SLAB_W, S], bf16)
            for q0, q1, e in ((0, 1, nc.sync), (1, 3, nc.sync), (3, 6, nc.scalar)):
                e.dma_start(s_slab[:, q0:q1, :, :], slab_d.ap()[:, q0:q1, :, :])
            s_xt = sb.tile([128, 8, HW], bf16)
            s_w1vr = sb.tile([128, 8, C], bf16)

            # --- phase A: T3[(po,par,dx), (w',t)] = weff3^T @ slab, one
            # matmul per (pair, t-half); copies transpose to t-major ---
            s_T = sb.tile([M3, S, NPAIR, SLAB_W], bf16)
            for q in range(NPAIR):
                for tau in range(2):
                    p_t = ps.tile([M3, SLAB_W, 8], f32, tag="pt")
                    nc.tensor.matmul(
                        p_t,
                        s_weff,
                        s_slab[:, q, :, 8 * tau : 8 * tau + 8],
                        start=True,
                        stop=True,
                    )
                    dst = s_T[:, 8 * tau : 8 * tau + 8, q, :]
                    if tau == 0:
                        nc.scalar.copy(dst, p_t.transpose([0, 2, 1]))
                    else:
                        nc.vector.tensor_copy(dst, p_t.transpose([0, 2, 1]))

            # --- T3 to DRAM: 3 po-class writes of po-shifted 4-pair windows,
            # each followed by its two (par) 3-dim batched gathers into
            # R[(par,t,q0), (po,dx), w].  Write po depends on copies for
            # pairs po..po+3 only, so early po classes pipeline behind the
            # conv; a partial tap-reduce runs per po as it lands. ---
            s_R = sb.tile([128, NTAP3, W], bf16)
            s_par = sb.tile([128, NPO, W], f32)
            g_engs = (nc.sync, nc.scalar, nc.sync)
            for po in range(NPO):
                e = g_engs[po]
                e.dma_start(
                    td_d.ap()[10 * po : 10 * po + 10],
                    s_T[10 * po : 10 * po + 10, :, po : po + 4, :],
                )
                other = nc.scalar if e is nc.sync else nc.sync
                for par in range(2):
                    src = bass.AP(
                        tensor=td_d.ap().tensor,
                        offset=(10 * po + 5 * par) * PLANE,
                        ap=[[SLAB_W, 64], [PLANE + 1, KS], [1, W]],
                    )
                    (e if par == 0 else other).dma_start(
                        s_R[64 * par : 64 * par + 64, KS * po : KS * po + KS, :],
                        src,
                    )
                nc.vector.tensor_reduce(
                    s_par[:, po, :],
                    s_R[:, KS * po : KS * po + KS, :].transpose([0, 2, 1]),
                    axis=mybir.AxisListType.X,
                    op=ALU.add,
                )

            # xt/w1vr on the HWDGE rings behind the gathers: they land during
            # the softmax phase, well before xattn/V need them
            nc.sync.dma_start(s_xt, xt_d.ap())
            nc.scalar.dma_start(s_w1vr, w1vr_d.ap())

            # --- final 3-way add of the partials -> sK[(par,t,q0), w] ---
            s_sk = sb.tile([128, W], f32)
            nc.vector.tensor_reduce(
                s_sk, s_par.transpose([0, 2, 1]), axis=mybir.AxisListType.X, op=ALU.add
            )

            # --- e = exp(sK); den via indicator-matmul on PE ---
            s_e16 = sb.tile([128, W], bf16)
            nc.scalar.activation(s_e16, s_sk, AF.Exp)
            p_den = pso.tile([ROWS, W], f32, tag="den")
            nc.tensor.matmul(p_den, s_hsel, s_e16, start=True, stop=True)
            s_rcp = sb.tile([ROWS, W], f32)
            nc.vector.reciprocal(s_rcp, p_den)
            nc.gpsimd.dma_start(dn_d.ap(), s_rcp)

            # --- replicate e to the (ci8,t)-partition layout on-chip:
            # mask-multiply expands e16 to the partition-diagonal
            # [128, (h', w)] tile, then one indicator matmul emits
            # eb[(rep,t), (h,w)] in PSUM (no DRAM bounce). ---
            s_ed = sb.tile([128, ROWS, W], bf16)
            nc.vector.tensor_tensor(
                s_ed,
                s_e16.unsqueeze(1).broadcast_to((128, ROWS, W)),
                s_hmask.unsqueeze(2).broadcast_to((128, ROWS, W)),
                op=ALU.mult,
            )
            p_eb = pso.tile([128, HW], f32, tag="eb")
            nc.tensor.matmul(
                p_eb, s_esel, s_ed.rearrange("p a b -> p (a b)"), start=True, stop=True
            )
            s_eb = sb.tile([128, HW], bf16)
            nc.scalar.copy(s_eb, p_eb)

            # --- V path: xattn = x_t * e in four chunks so the V matmuls
            # pipeline tightly behind the multiplies ---
            s_xa = sb.tile([128, 8, HW], bf16)
            ebb = s_eb.unsqueeze(1).broadcast_to((128, 2, HW))
            p_o = pso.tile([C, HW], f32, tag="out")
            for qq in range(4):
                nc.vector.tensor_tensor(
                    s_xa[:, 2 * qq : 2 * qq + 2, :],
                    s_xt[:, 2 * qq : 2 * qq + 2, :],
                    ebb,
                    op=ALU.mult,
                )
            for g in range(8):
                nc.tensor.matmul(
                    p_o,
                    s_w1vr[:, g, :],
                    s_xa[:, g, :],
                    start=(g == 0),
                    stop=(g == 7),
                )
            # raw (unnormalized) output; the host divides by den via dn
            s_o = sb.tile([C, HW], bf16)
            nc.vector.tensor_copy(s_o, p_o)
            nc.sync.dma_start(o_d.ap(), s_o)

    nc.compile()
    return nc


def _get_module():
    global _MODULE
    if _MODULE is None:
        _MODULE = _build_module()
    return _MODULE


def make_host_inputs(x, w1, b1, w2, b2):
    """Host-side precompute: folded weights + per-core reflect-padded slices."""
    x = np.ascontiguousarray(np.asarray(x, np.float32))
    w1 = np.asarray(w1, np.float32)
    w2 = np.asarray(w2, np.float32)

    w1K = w1[C : 2 * C, :, 0, 0]          # [c, ci]
    w2K = w2[0, C : 2 * C]                # [c, 5, 5]
    weff = np.einsum("ci,cyx->iyx", w1K, w2K)  # [ci, dy, dx]
    # weff3[(ci,rp), (po,par,dx)] = weff[ci, 2po+rp-par, dx] (OOR dy -> 0)
    weff3 = np.zeros((128, M3), np.float32)
    for rp in range(2):
        for po in range(NPO):
            for par in range(2):
                dy = 2 * po + rp - par
                if 0 <= dy < KS:
                    for dx in range(KS):
                        weff3[2 * np.arange(C) + rp, 10 * po + 5 * par + dx] = weff[
                            :, dy, dx
                        ]
    weff3 = weff3.astype(BF16)
    w1V = w1[2 * C :, :, 0, 0]            # [co, ci]

    # w1vr[(ci8,t), g, co] = w1V[co, 8g+ci8]
    tmp = w1V.T.reshape(8, 8, C)                      # (g, ci8, co)
    w1vr = np.ascontiguousarray(
        np.broadcast_to(tmp[:, :, None, :], (8, 8, S, C))
        .transpose(1, 2, 0, 3)
        .reshape(128, 8, C)
    ).astype(BF16)

    # sK partition p = 64*par + 4*t + q0;  h(p) = 2*q0 + par;  t(p)
    pp = np.arange(128)
    par_p, t_p, q0_p = pp // 64, (pp % 64) // 4, pp % 4
    h_p = 2 * q0_p + par_p

    # hsel[p, m] = [m == h(p)]   (den indicator + diagonal h-mask)
    hsel = np.zeros((128, ROWS), np.float32)
    hsel[pp, h_p] = 1.0
    hsel = hsel.astype(BF16)
    hmask = hsel

    # esel[p, rep*S + t'] = [t' == t(p)]
    esel = np.zeros((128, 128), np.float32)
    for rep in range(8):
        esel[pp, rep * S + t_p] = 1.0
    esel = esel.astype(BF16)

    cst = np.concatenate([weff3, hsel, hmask, esel], axis=1)
    assert cst.shape == (128, CONSTW)

    in_maps = []
    for core in range(NCORES):
        b, hc = divmod(core, 4)
        h0 = ROWS * hc
        xp = np.pad(x[b], ((0, 0), (PAD, PAD), (PAD, PAD), (0, 0)), mode="reflect")
        sl = xp[:, h0 : h0 + SLAB_R, :, :]            # [ci, r, w36, t]
        # slab[(ci,rp), q, w', t] = sl[ci, 2q+rp, w', t]
        slab = np.ascontiguousarray(
            sl.reshape(C, NPAIR, 2, SLAB_W, S)
            .transpose(0, 2, 1, 3, 4)
            .reshape(128, NPAIR, SLAB_W, S)
        ).astype(BF16)
        xs = x[b][:, h0 : h0 + ROWS, :, :]            # [ci, h, w, t]
        xt = np.ascontiguousarray(
            xs.reshape(8, 8, ROWS, W, S)
            .transpose(1, 4, 0, 2, 3)
            .reshape(128, 8, HW)
        ).astype(BF16)
        in_maps.append(
            {"slab": slab, "xt": xt, "cst": cst, "w1vr": w1vr}
        )
    return in_maps


def assemble_output(results, b1):
    b1V = np.asarray(b1, np.float32)[2 * C :]
    out = np.empty((B, C, H, W, S), np.float32)
    for core in range(NCORES):
        b, hc = divmod(core, 4)
        h0 = ROWS * hc
        r = results[core]
        o = r["o"].astype(np.float32).reshape(C, ROWS, W) * r["dn"].reshape(
            1, ROWS, W
        )
        out[b, :, h0 : h0 + ROWS, :, :] = o[:, :, :, None]
    out += b1V[None, :, None, None, None]
    return out


def kernel(x, w1, b1, w2, b2):
    from concourse.bass_utils import run_bass_kernel_spmd

    nc = _get_module()
    in_maps = make_host_inputs(x, w1, b1, w2, b2)
    res = run_bass_kernel_spmd(nc, in_maps, core_ids=list(range(NCORES)))
    return assemble_output(res.results, b1)
